# revision 16
# baseline (speedup 1.0000x reference)
"""Causal RBF (non-softmax) attention on 8 Trainium2 NeuronCores.

Problem: q,k,v [B=2, H=16, N=2048, D=128] f32.
  logits = 2s*q@k^T - s*||q||^2 - s*||k||^2   (s = 1/sqrt(D))
  p = exp(logits) with causal mask; out = p @ v      (no softmax normalization)

Sharding: B*H = 32 heads -> 4 heads per core, fully independent.

Algebra used to make the device kernel cheap:
  out[m,:] = eq[m] * sum_{n<=m} exp(2s*q_m.k_n) * (ek[n]*v[n,:])
  with eq[m] = exp(-s*||q_m||^2), ek[n] = exp(-s*||k_n||^2).
The host folds 2s into qT, ek into v, and applies eq to the output rows, so
the device computes only:  ST = KT^T.T @ QT blocks -> Exp -> mask -> @ V'.

Device layouts (per head):
  qT [128(d), 2048(m)]  (host-transposed, scaled by 2s)
  kT [128(d), 2048(n)]  (host-transposed)
  v' [2048(n), 128(d)]  (ek-scaled, natural)
Output is written transposed, OT [128(d), 2048(m)]; host transposes back.

Matmuls run as float32r (full PE rate at free-dim >= 256; ~3e-4 rel err from
its tf32-like rounding). PSUM: ST group tiles [128, 3, 512] double-buffered
(6 banks) + per-supertile OT accumulator tiles (2 banks); PV matmuls
accumulate straight into OT. Emission is software-pipelined one group ahead
so the PE always has queued work while ACT runs Exp. Measured on 8 axon trn2
cores: ~98.6 us NEFF exec, rel err 2.9e-4 (ACT/exp-roofline bound: 71 us of
pure EXP at 1 elem/lane/cycle is irreducible for causal N=2048 x 4 heads).
"""

import math
import sys
import time

import numpy as np

sys.path.insert(0, "/opt/trn_rl_repo")

import concourse.mybir as mybir
import concourse.tile as tile
from concourse import bacc, bass_utils

F32 = mybir.dt.float32
F32R = mybir.dt.float32r
EXP = mybir.ActivationFunctionType.Exp

B, H, N, D = 2, 16, 2048, 128
SM = 1.0 / math.sqrt(D)
P = 128
NCORES = 8
HPC = (B * H) // NCORES  # heads per core
MW = 512                 # m (query) super-tile width
G = 4                    # k-blocks per group (4 x 512 f32 = 4 PSUM banks)


def _emit_body(tc, qt, kt, v, cmask, out, hpc, n):
    """Software-pipelined emission: for the flat list of (supertile, group)
    work items, group k+1's ST matmuls + Exp are emitted BEFORE group k's
    masks/PV matmuls, so the scheduler always has PE work queued while ACT
    or DVE finish the previous group. Head h+1's chunked loads are emitted
    at the start of head h (a full head of DMA lead time).

    Per group: 3 ST matmuls (f32r, 3 PSUM banks) -> one Exp -> PV matmuls
    accumulating straight into a per-supertile PSUM OT tile. The diagonal
    4-block group packs narrowed matmuls into 3 banks:
      bank0 = t0 [m 0:512); bank1 = t2 | t3 (both [m 256:512));
      bank2 = t1 [m 128:512) | 128 unused cols (exp covers flat [0,1408)).
    Causal masking = tiny DVE multiplies on 128-col triangles only.
    """
    nc = tc.nc
    mi = n // MW    # query super tiles per head
    from contextlib import ExitStack

    with ExitStack() as ctx:
        const = ctx.enter_context(tc.tile_pool(name="const", bufs=1))
        qk_pool = ctx.enter_context(tc.tile_pool(name="qk", bufs=3))
        v_pool = ctx.enter_context(tc.tile_pool(name="vp", bufs=3))
        st_pool = ctx.enter_context(tc.tile_pool(name="st", bufs=2, space="PSUM"))
        otp_pool = ctx.enter_context(tc.tile_pool(name="otp", bufs=2, space="PSUM"))
        pt_pool = ctx.enter_context(tc.tile_pool(name="pt", bufs=5))
        osb_pool = ctx.enter_context(tc.tile_pool(name="osb", bufs=3))

        # cmask [P, 256] = [zeros(128) | upper-tri(128)]; tri = cols 128:256
        cm_sb = const.tile([P, 2 * P], F32R)
        nc.sync.dma_start(cm_sb[:], cmask[:])
        ztri = cm_sb[:, 0 : 2 * P]
        tri = cm_sb[:, P : 2 * P]

        head_tiles = {}

        def emit_loads(h):
            qt_c, kt_c, v_c = [], [], []
            for c in range(mi):
                qtc = qk_pool.tile([P, MW], F32R, tag=f"qt{c}")
                ktc = qk_pool.tile([P, MW], F32R, tag=f"kt{c}")
                vc = v_pool.tile([P, G, P], F32R, tag=f"v{c}")
                nc.sync.dma_start(qtc[:], qt[h, :, c * MW : (c + 1) * MW])
                nc.sync.dma_start(ktc[:], kt[h, :, c * MW : (c + 1) * MW])
                nc.sync.dma_start(
                    vc[:],
                    v[h, c * G * P : (c + 1) * G * P].rearrange(
                        "(nb p) d -> p nb d", p=P
                    ),
                )
                qt_c.append(qtc)
                kt_c.append(ktc)
                v_c.append(vc)
            head_tiles[h] = (qt_c, kt_c, v_c)

        # flat work list: (h, i, chunk-of-k-blocks-or-"diag", is_last_group)
        work = []
        for h in range(hpc):
            for i in range(mi):
                fullb = list(range(4 * i))
                for c0 in range(0, len(fullb), 3):
                    work.append((h, i, fullb[c0 : c0 + 3], False))
                work.append((h, i, None, True))  # diag group

        ustate = {}  # (h,i) -> dict(ot=..., first=...)
        pend = {}    # k -> (st, pt) tiles

        def kt_blk(h, j):
            return head_tiles[h][1][j // G][:, (j % G) * P : (j % G + 1) * P]

        def v_blk(h, j):
            return head_tiles[h][2][j // G][:, j % G, :]

        def st_exp(k):
            h, i, chunk, isdiag_last = work[k]
            if i == 1 and chunk is not None and chunk[:1] == [0] and h + 1 < hpc:
                # early in head h: prefetch head h+1's tensors
                emit_loads(h + 1)
            qs = head_tiles[h][0][i][:]
            st = st_pool.tile([P, 3, MW], F32, tag="st")
            pt = pt_pool.tile([P, 3, MW], F32R, tag="pt")
            if chunk is not None:
                for idx, j in enumerate(chunk):
                    nc.tensor.matmul(
                        st[:, idx, :], lhsT=kt_blk(h, j),
                        rhs=qs, start=True, stop=True,
                    )
                nc.scalar.activation(
                    pt[:, : len(chunk), :], st[:, : len(chunk), :], EXP
                )
            else:
                jb = 4 * i
                nc.tensor.matmul(st[:, 0, :], lhsT=kt_blk(h, jb),
                                 rhs=qs, start=True, stop=True)
                nc.tensor.matmul(st[:, 1, 0:256], lhsT=kt_blk(h, jb + 2),
                                 rhs=qs[:, 256:512], start=True, stop=True)
                nc.tensor.matmul(st[:, 1, 256:512], lhsT=kt_blk(h, jb + 3),
                                 rhs=qs[:, 256:512], start=True, stop=True)
                nc.tensor.matmul(st[:, 2, 0:384], lhsT=kt_blk(h, jb + 1),
                                 rhs=qs[:, 128:512], start=True, stop=True)
                st_flat = st.rearrange("p a b -> p (a b)")
                pt_flat = pt.rearrange("p a b -> p (a b)")
                nc.scalar.activation(pt_flat[:, 0:1408], st_flat[:, 0:1408], EXP)
            pend[k] = (st, pt)

        def finish(k):
            h, i, chunk, islast = work[k]
            st, pt = pend.pop(k)
            u = ustate.get((h, i))
            if u is None:
                ot_tile = otp_pool.tile([P, MW], F32, tag="otp", name="ot_tile")
                u = ustate[(h, i)] = {"ot": ot_tile, "first": True}
            ot = u["ot"]

            def pv(j, rhs, osl, stop=False):
                nc.tensor.matmul(osl, lhsT=v_blk(h, j), rhs=rhs,
                                 start=u["first"], stop=stop)
                u["first"] = False

            if chunk is not None:
                for idx, j in enumerate(chunk):
                    pv(j, pt[:, idx, :], ot[:, :])
            else:
                jb = 4 * i
                nc.vector.tensor_mul(pt[:, 0, 0:P], pt[:, 0, 0:P], tri)
                nc.vector.tensor_mul(pt[:, 2, 0:P], pt[:, 2, 0:P], tri)
                nc.vector.tensor_mul(pt[:, 1, 0:P], pt[:, 1, 0:P], tri)
                nc.vector.tensor_mul(pt[:, 1, 256:512], pt[:, 1, 256:512], ztri)
                pv(jb + 0, pt[:, 0, :], ot[:, :])
                pv(jb + 1, pt[:, 2, 0:384], ot[:, 128:512])
                pv(jb + 2, pt[:, 1, 0:256], ot[:, 256:512])
                pv(jb + 3, pt[:, 1, 256:512], ot[:, 256:512], stop=True)
                # close out the supertile
                out_sb = osb_pool.tile([P, MW], F32, tag="osb")
                nc.vector.tensor_copy(out_sb[:], ot[:])
                nc.gpsimd.dma_start(out[h, :, i * MW : (i + 1) * MW], out_sb[:])

        emit_loads(0)
        st_exp(0)
        if len(work) > 1:
            st_exp(1)
        for k in range(len(work)):
            if k + 2 < len(work):
                st_exp(k + 2)
            finish(k)


def _build(hpc=HPC, n=N):
    nc = bacc.Bacc(
        "TRN2", target_bir_lowering=False, debug=False, num_devices=NCORES
    )
    qt = nc.dram_tensor("qt", [hpc, P, n], F32R, kind="ExternalInput").ap()
    kt = nc.dram_tensor("kt", [hpc, P, n], F32R, kind="ExternalInput").ap()
    v = nc.dram_tensor("v", [hpc, n, P], F32R, kind="ExternalInput").ap()
    cmask = nc.dram_tensor("cmask", [P, 2 * P], F32R, kind="ExternalInput").ap()
    out = nc.dram_tensor("out", [hpc, P, n], F32, kind="ExternalOutput").ap()
    with tile.TileContext(nc) as tc:
        _emit_body(tc, qt, kt, v, cmask, out, hpc, n)
    nc.compile()
    return nc


_NC_CACHE = {}


def _get_nc():
    if "nc" not in _NC_CACHE:
        _NC_CACHE["nc"] = _build()
    return _NC_CACHE["nc"]


def _make_mask():
    # cmask [P, 256] = [zeros(128) | tri(128)], tri[p, c] = 1 where c >= p
    z = np.zeros((P, P), dtype=np.float32)
    c = np.arange(P)[None, :]
    p = np.arange(P)[:, None]
    tri = (c >= p).astype(np.float32)
    return np.concatenate([z, tri], axis=1)


def _prep(q, k, v):
    """Host-side reshaping/folding. Returns per-core in_maps and eq for post."""
    q = np.asarray(q, dtype=np.float32).reshape(B * H, N, D)
    k = np.asarray(k, dtype=np.float32).reshape(B * H, N, D)
    v = np.asarray(v, dtype=np.float32).reshape(B * H, N, D)

    qT = np.ascontiguousarray(q.transpose(0, 2, 1)) * np.float32(2.0 * SM)
    kT = np.ascontiguousarray(k.transpose(0, 2, 1))
    ek = np.exp(np.float32(-SM) * np.einsum("hnd,hnd->hn", k, k)).astype(np.float32)
    eq = np.exp(np.float32(-SM) * np.einsum("hnd,hnd->hn", q, q)).astype(np.float32)
    vs = (v * ek[:, :, None]).astype(np.float32)

    mask = _make_mask()
    in_maps = []
    for c in range(NCORES):
        s = slice(c * HPC, (c + 1) * HPC)
        in_maps.append(
            {
                "qt": np.ascontiguousarray(qT[s]),
                "kt": np.ascontiguousarray(kT[s]),
                "v": np.ascontiguousarray(vs[s]),
                "cmask": mask,
            }
        )
    return in_maps, eq


def _run(in_maps, trace=False):
    nc = _get_nc()
    res = bass_utils.run_bass_kernel_spmd(
        nc, in_maps, core_ids=list(range(NCORES)), trace=trace
    )
    return res


def _post(res_list, eq):
    # res_list: per-core dicts with "out" [HPC, 128(d), N(m)]
    ot = np.concatenate([r["out"] for r in res_list], axis=0)  # [B*H, D, N]
    o = ot.transpose(0, 2, 1) * eq[:, :, None]  # [B*H, N, D]
    return np.ascontiguousarray(o.reshape(B, H, N, D).astype(np.float32))


def kernel(q, k, v):
    in_maps, eq = _prep(q, k, v)
    last_err = None
    for attempt in range(3):
        try:
            res = _run(in_maps, trace=False)
            return _post(res.results, eq)
        except Exception as e:  # axon/NRT first-run flakiness: retry
            last_err = e
            time.sleep(2.0)
    raise last_err
